# revision 4
# baseline (speedup 1.0000x reference)
"""Trainium2 Bass kernel for nn_MoETransformerBlock_68728066670794.

Strategy (8 NeuronCores):
  Phase A - attention, sharded by (batch, kv-group): core i -> b=i//4, g=i%4.
    Each core computes its 3 query heads (GQA rep=3) of one batch and the
    partial output projection over its 192 Wo rows. Host sums partials.
    Scores are computed transposed (S^T[k,q]) so no on-device transposes are
    needed; softmax uses exp without max-subtraction (scores are O(1) here)
    and the denominator comes from a ones-column appended to V.
    RoPE is applied via a second projection with a permuted/negated weight
    matrix (rot(q) = h @ Wq_rot), so it becomes pure elementwise math in the
    transposed layout.
  Host - routing (rmsnorm, gate logits, top-2, aux loss) in numpy.
  Phase B - MoE, expert-parallel: core e gets expert e's weights and the
    tokens routed to it (capacity-padded). Computes the gated FFN
    (silu(x@W1) * (x@W3)) @ W2, scaled by combine weights. Host scatter-adds.

All matmuls run as float32r (TF32-like, full PE rate; ~1e-4 rel err).
"""

import os
import sys

sys.path.insert(0, "/opt/trn_rl_repo")

from contextlib import ExitStack

import numpy as np

import concourse.bass as bass
import concourse.tile as tile
from concourse import bacc, mybir
from concourse.bass_utils import run_bass_kernel_spmd
from concourse.masks import make_identity

F32 = mybir.dt.float32
F32R = mybir.dt.float32r
AF = mybir.ActivationFunctionType

B, T, C = 2, 1024, 768
H, HK, HD = 12, 4, 64
REP = H // HK  # 3 query heads per kv head
E, FF = 8, 3072
N_CORES = 8
EPS = 1e-6

_cache = {}

# Collected HW exec times (ns) when KERNEL_TRACE=1 (one entry per launch).
LAST_EXEC_NS = []


def _trace_on():
    return os.environ.get("KERNEL_TRACE", "0") == "1"


def _run(nc, in_maps, label):
    kw = {}
    if _trace_on():
        kw["trace"] = True
    res = run_bass_kernel_spmd(nc, in_maps, list(range(N_CORES)), **kw)
    if _trace_on():
        LAST_EXEC_NS.append((label, res.exec_time_ns))
    return res.results


# ---------------------------------------------------------------------------
# Phase A: attention
# ---------------------------------------------------------------------------

def _build_attn():
    if "attn" in _cache:
        return _cache["attn"]
    nc = bacc.Bacc("TRN2", target_bir_lowering=False, debug=False,
                   num_devices=N_CORES)
    ht = nc.dram_tensor("ht", [C, T], F32, kind="ExternalInput").ap()
    cosT = nc.dram_tensor("cosT", [HD, T], F32, kind="ExternalInput").ap()
    sinT = nc.dram_tensor("sinT", [HD, T], F32, kind="ExternalInput").ap()
    wq = nc.dram_tensor("wq", [C, REP * HD], F32, kind="ExternalInput").ap()
    wqr = nc.dram_tensor("wqr", [C, REP * HD], F32, kind="ExternalInput").ap()
    wk = nc.dram_tensor("wk", [C, HD], F32, kind="ExternalInput").ap()
    wkr = nc.dram_tensor("wkr", [C, HD], F32, kind="ExternalInput").ap()
    wv = nc.dram_tensor("wv", [C, HD], F32, kind="ExternalInput").ap()
    wo3 = nc.dram_tensor("wo3", [REP, HD, C], F32, kind="ExternalInput").ap()
    out = nc.dram_tensor("attn_out", [T, C], F32, kind="ExternalOutput").ap()

    NCT = C // 128  # 6 contraction tiles
    NTT = T // 128  # 8 token tiles

    with tile.TileContext(nc, trace_sim=False) as tc, ExitStack() as ctx:
        cst = ctx.enter_context(tc.tile_pool(name="cst", bufs=1))
        sbw = ctx.enter_context(tc.tile_pool(name="sbw", bufs=3))
        sbe = ctx.enter_context(tc.tile_pool(name="sbe", bufs=9))
        ps_proj = ctx.enter_context(
            tc.tile_pool(name="ps_proj", bufs=2, space="PSUM"))
        ps_s = ctx.enter_context(
            tc.tile_pool(name="ps_s", bufs=2, space="PSUM"))
        ps_o = ctx.enter_context(
            tc.tile_pool(name="ps_o", bufs=1, space="PSUM"))

        hsb = cst.tile([128, NCT, T], F32R, tag="hsb")
        nc.sync.dma_start(
            hsb[:], ht.bitcast(F32R).rearrange("(k p) t -> p k t", p=128))
        cos_sb = cst.tile([HD, T], F32, tag="cos")
        nc.sync.dma_start(cos_sb[:], cosT)
        sin_sb = cst.tile([HD, T], F32, tag="sin")
        nc.sync.dma_start(sin_sb[:], sinT)
        wq_sb = cst.tile([128, NCT, REP * HD], F32R, tag="wq")
        nc.sync.dma_start(
            wq_sb[:], wq.bitcast(F32R).rearrange("(k p) d -> p k d", p=128))
        wqr_sb = cst.tile([128, NCT, REP * HD], F32R, tag="wqr")
        nc.sync.dma_start(
            wqr_sb[:], wqr.bitcast(F32R).rearrange("(k p) d -> p k d", p=128))
        wk_sb = cst.tile([128, NCT, HD], F32R, tag="wk")
        nc.sync.dma_start(
            wk_sb[:], wk.bitcast(F32R).rearrange("(k p) d -> p k d", p=128))
        wkr_sb = cst.tile([128, NCT, HD], F32R, tag="wkr")
        nc.sync.dma_start(
            wkr_sb[:], wkr.bitcast(F32R).rearrange("(k p) d -> p k d", p=128))
        wv_sb = cst.tile([128, NCT, HD], F32R, tag="wv")
        nc.sync.dma_start(
            wv_sb[:], wv.bitcast(F32R).rearrange("(k p) d -> p k d", p=128))
        wo_sb = []
        for h in range(REP):
            w = cst.tile([HD, C], F32R, tag=f"wo{h}")
            nc.sync.dma_start(w[:], wo3[h].bitcast(F32R))
            wo_sb.append(w)

        ident = cst.tile([128, 128], F32, tag="ident")
        make_identity(nc, ident[:])
        ones_t = cst.tile([128, 1], F32, tag="ones")
        nc.gpsimd.memset(ones_t[:], 1.0)

        # ---- V (natural layout, with ones column) ----
        vT_sb = cst.tile([HD, T], F32, tag="vT")
        for ch in range(2):
            sl = slice(ch * 512, (ch + 1) * 512)
            psv = ps_proj.tile([HD, 512], F32, tag="proj")
            for c in range(NCT):
                nc.tensor.matmul(psv[:], wv_sb[:, c, :], hsb[:, c, sl],
                                 start=(c == 0), stop=(c == NCT - 1))
            nc.scalar.copy(vT_sb[:, sl], psv[:])
        v1 = []
        for tt in range(NTT):
            pst = ps_proj.tile([128, HD], F32, tag="proj")
            nc.tensor.transpose(
                pst[:], vT_sb[:, tt * 128:(tt + 1) * 128], ident[0:HD, 0:HD])
            vt = cst.tile([128, HD + 1], F32R, tag=f"v1_{tt}")
            nc.scalar.copy(vt[:, 0:HD], pst[:])
            nc.scalar.copy(vt[:, HD:HD + 1], ones_t[:])
            v1.append(vt)

        # ---- K with rope ----
        krope = cst.tile([HD, T], F32R, tag="krope")
        for ch in range(2):
            sl = slice(ch * 512, (ch + 1) * 512)
            psk = ps_proj.tile([HD, 512], F32, tag="proj")
            pskr = ps_proj.tile([HD, 512], F32, tag="proj")
            for c in range(NCT):
                nc.tensor.matmul(psk[:], wk_sb[:, c, :], hsb[:, c, sl],
                                 start=(c == 0), stop=(c == NCT - 1))
            for c in range(NCT):
                nc.tensor.matmul(pskr[:], wkr_sb[:, c, :], hsb[:, c, sl],
                                 start=(c == 0), stop=(c == NCT - 1))
            t1 = sbw.tile([HD, 512], F32, tag="ropetmp")
            t2 = sbw.tile([HD, 512], F32, tag="ropetmp")
            nc.vector.tensor_mul(t1[:], psk[:], cos_sb[:, sl])
            nc.vector.tensor_mul(t2[:], pskr[:], sin_sb[:, sl])
            nc.vector.tensor_add(krope[:, sl], t1[:], t2[:])

        # ---- per head: q rope, scores, softmax, O ----
        oT = []
        for h in range(REP):
            hsl = slice(h * HD, (h + 1) * HD)
            qrope = cst.tile([HD, T], F32R, tag=f"qrope{h}")
            for ch in range(2):
                sl = slice(ch * 512, (ch + 1) * 512)
                psq = ps_proj.tile([HD, 512], F32, tag="proj")
                psqr = ps_proj.tile([HD, 512], F32, tag="proj")
                for c in range(NCT):
                    nc.tensor.matmul(psq[:], wq_sb[:, c, hsl], hsb[:, c, sl],
                                     start=(c == 0), stop=(c == NCT - 1))
                for c in range(NCT):
                    nc.tensor.matmul(psqr[:], wqr_sb[:, c, hsl],
                                     hsb[:, c, sl],
                                     start=(c == 0), stop=(c == NCT - 1))
                t1 = sbw.tile([HD, 512], F32, tag="ropetmp")
                t2 = sbw.tile([HD, 512], F32, tag="ropetmp")
                nc.vector.tensor_mul(t1[:], psq[:], cos_sb[:, sl])
                nc.vector.tensor_mul(t2[:], psqr[:], sin_sb[:, sl])
                nc.vector.tensor_add(qrope[:, sl], t1[:], t2[:])

            # scores (transposed) + exp + causal mask
            e_tiles = []
            for i in range(NTT):
                et = sbe.tile([128, T], F32R, tag="e")
                ksl = slice(i * 128, (i + 1) * 128)
                if i < 4:
                    w_lo = 512 - i * 128
                    ps_lo = ps_s.tile([128, w_lo], F32, tag="slo")
                    nc.tensor.matmul(ps_lo[:], krope[:, ksl],
                                     qrope[:, i * 128:512],
                                     start=True, stop=True)
                    nc.scalar.activation(et[:, i * 128:512], ps_lo[:],
                                         AF.Exp, scale=0.125)
                ps_hi = ps_s.tile([128, 512], F32, tag="shi")
                qs = max(512, i * 128)
                nc.tensor.matmul(ps_hi[:, qs - 512:512], krope[:, ksl],
                                 qrope[:, qs:1024], start=True, stop=True)
                nc.scalar.activation(et[:, qs:1024], ps_hi[:, qs - 512:512],
                                     AF.Exp, scale=0.125)
                # causal: keep q >= k within the diagonal block
                nc.gpsimd.affine_select(
                    out=et[:, ksl], in_=et[:, ksl],
                    compare_op=mybir.AluOpType.is_ge, fill=0.0,
                    base=0, pattern=[[1, 128]], channel_multiplier=-1)
                e_tiles.append(et)

            pso = ps_o.tile([HD + 1, T], F32, tag="o")
            # bank 0: q in [0, 512)
            for i in range(4):
                nc.tensor.matmul(pso[:, i * 128:512], v1[i][:],
                                 e_tiles[i][:, i * 128:512],
                                 start=(i == 0), stop=(i == 3))
            # bank 1: q in [512, 1024)
            for i in range(NTT):
                qs = max(512, i * 128)
                nc.tensor.matmul(pso[:, qs:1024], v1[i][:],
                                 e_tiles[i][:, qs:1024],
                                 start=(i == 0), stop=(i == NTT - 1))
            lrow = sbw.tile([1, T], F32, tag="lrow")
            nc.scalar.copy(lrow[:], pso[HD:HD + 1, :])
            rinv = sbw.tile([1, T], F32, tag="rinv")
            nc.vector.reciprocal(rinv[:], lrow[:])
            rlb = sbw.tile([HD, T], F32, tag="rlb")
            nc.gpsimd.partition_broadcast(rlb[:], rinv[:], channels=HD)
            ot = cst.tile([HD, T], F32R, tag=f"oT{h}")
            for ch in range(2):
                sl = slice(ch * 512, (ch + 1) * 512)
                nc.vector.tensor_mul(ot[:, sl], pso[0:HD, sl], rlb[:, sl])
            oT.append(ot)

        # ---- output projection (partial Wo rows) ----
        for tt in range(NTT):
            tsl = slice(tt * 128, (tt + 1) * 128)
            pwa = ps_s.tile([128, 512], F32, tag="slo")
            pwb = ps_s.tile([128, 256], F32, tag="shi")
            for h in range(REP):
                nc.tensor.matmul(pwa[:], oT[h][:, tsl], wo_sb[h][:, 0:512],
                                 start=(h == 0), stop=(h == REP - 1))
            for h in range(REP):
                nc.tensor.matmul(pwb[:], oT[h][:, tsl], wo_sb[h][:, 512:768],
                                 start=(h == 0), stop=(h == REP - 1))
            osa = sbw.tile([128, 512], F32, tag="osa")
            osb = sbw.tile([128, 256], F32, tag="osb")
            nc.scalar.copy(osa[:], pwa[:])
            nc.vector.tensor_copy(osb[:], pwb[:])
            nc.sync.dma_start(out[tsl, 0:512], osa[:])
            nc.sync.dma_start(out[tsl, 512:768], osb[:])

    nc.compile()
    _cache["attn"] = nc
    return nc


# ---------------------------------------------------------------------------
# Phase B: MoE expert FFN
# ---------------------------------------------------------------------------

def _pieces(cap):
    """Decompose cap into free-dim pieces (prefer >=256 for full-rate f32r)."""
    out, r, off = [], cap, 0
    while r > 0:
        if r <= 512:
            ln = r
        elif r - 512 >= 256:
            ln = 512
        else:
            ln = 384
        out.append((off, ln))
        off += ln
        r -= ln
    return out

def _build_moe(cap):
    key = ("moe", cap)
    if key in _cache:
        return _cache[key]
    nc = bacc.Bacc("TRN2", target_bir_lowering=False, debug=False,
                   num_devices=N_CORES)
    xgT = nc.dram_tensor("xgT", [C, cap], F32, kind="ExternalInput").ap()
    cw2 = nc.dram_tensor("cw2", [128, cap // 128], F32,
                         kind="ExternalInput").ap()
    w1r = nc.dram_tensor("w1r", [FF // 128, C, 128], F32,
                         kind="ExternalInput").ap()
    w3r = nc.dram_tensor("w3r", [FF // 128, C, 128], F32,
                         kind="ExternalInput").ap()
    w2 = nc.dram_tensor("w2", [FF, C], F32, kind="ExternalInput").ap()
    out = nc.dram_tensor("moe_out", [cap, C], F32, kind="ExternalOutput").ap()

    NCT = C // 128   # 6
    NFT = FF // 128  # 24
    NTT = cap // 128
    pieces = _pieces(cap)

    with tile.TileContext(nc, trace_sim=False) as tc, ExitStack() as ctx:
        cst = ctx.enter_context(tc.tile_pool(name="cst", bufs=1))
        sbw = ctx.enter_context(tc.tile_pool(name="sbw", bufs=3))
        psp = ctx.enter_context(tc.tile_pool(name="psp", bufs=2, space="PSUM"))

        xg_sb = cst.tile([128, NCT, cap], F32R, tag="xg")
        nc.sync.dma_start(
            xg_sb[:], xgT.bitcast(F32R).rearrange("(k p) t -> p k t", p=128))
        cw_sb = cst.tile([128, cap // 128], F32, tag="cw")
        nc.sync.dma_start(cw_sb[:], cw2)
        w2_sb = cst.tile([128, NFT, C], F32R, tag="w2")
        nc.sync.dma_start(
            w2_sb[:], w2.bitcast(F32R).rearrange("(f p) c -> p f c", p=128))

        aT = [cst.tile([128, cap], F32R, tag=f"aT{ft}", name=f"aT{ft}")
              for ft in range(NFT)]

        for ft in range(NFT):
            w1t = sbw.tile([128, NCT, 128], F32R, tag="w1t")
            nc.sync.dma_start(
                w1t[:],
                w1r[ft].bitcast(F32R).rearrange("(k p) m -> p k m", p=128))
            w3t = sbw.tile([128, NCT, 128], F32R, tag="w3t")
            nc.sync.dma_start(
                w3t[:],
                w3r[ft].bitcast(F32R).rearrange("(k p) m -> p k m", p=128))
            for pi, (poff, plen) in enumerate(pieces):
                psl = slice(poff, poff + plen)
                ps1 = psp.tile([128, plen], F32, tag=f"p1_{pi}")
                ps3 = psp.tile([128, plen], F32, tag=f"p3_{pi}")
                for c in range(NCT):
                    nc.tensor.matmul(ps1[:], w1t[:, c, :], xg_sb[:, c, psl],
                                     start=(c == 0), stop=(c == NCT - 1))
                for c in range(NCT):
                    nc.tensor.matmul(ps3[:], w3t[:, c, :], xg_sb[:, c, psl],
                                     start=(c == 0), stop=(c == NCT - 1))
                ss = sbw.tile([128, plen], F32, tag="silu")
                nc.scalar.activation(ss[:], ps1[:], AF.Silu)
                nc.vector.tensor_mul(aT[ft][:, psl], ss[:], ps3[:])

        for ct in range(NTT):
            tsl = slice(ct * 128, (ct + 1) * 128)
            pya = psp.tile([128, 512], F32, tag="p1_0")
            pyb = psp.tile([128, 256], F32, tag="p3_0")
            for ft in range(NFT):
                nc.tensor.matmul(pya[:], aT[ft][:, tsl], w2_sb[:, ft, 0:512],
                                 start=(ft == 0), stop=(ft == NFT - 1))
            for ft in range(NFT):
                nc.tensor.matmul(pyb[:], aT[ft][:, tsl], w2_sb[:, ft, 512:768],
                                 start=(ft == 0), stop=(ft == NFT - 1))
            ya = sbw.tile([128, 512], F32, tag="yout")
            yb = sbw.tile([128, 256], F32, tag="youtb")
            nc.vector.tensor_scalar_mul(ya[:], pya[:], cw_sb[:, ct:ct + 1])
            nc.vector.tensor_scalar_mul(yb[:], pyb[:], cw_sb[:, ct:ct + 1])
            nc.sync.dma_start(out[tsl, 0:512], ya[:])
            nc.sync.dma_start(out[tsl, 512:768], yb[:])

    nc.compile()
    _cache[key] = nc
    return nc


# ---------------------------------------------------------------------------
# Host orchestration
# ---------------------------------------------------------------------------

def _rmsnorm_np(x, w):
    ms = np.mean(x.astype(np.float64) ** 2, axis=-1, keepdims=True)
    return (x * (1.0 / np.sqrt(ms + EPS))).astype(np.float32) * w


def _rot_weights(w, n_heads):
    """wr such that h @ wr == rotate_half(h @ w) per head of width HD."""
    wr = np.empty_like(w)
    for j in range(n_heads):
        c = j * HD
        wr[:, c:c + HD // 2] = -w[:, c + HD // 2:c + HD]
        wr[:, c + HD // 2:c + HD] = w[:, c:c + HD // 2]
    return wr


def kernel(x, cos, sin, ln1_w, ln2_w, Wq, Wk, Wv, Wo, Wg, W1, W2, W3):
    x = np.asarray(x, dtype=np.float32)
    cos = np.asarray(cos, dtype=np.float32)
    sin = np.asarray(sin, dtype=np.float32)
    ln1_w = np.asarray(ln1_w, dtype=np.float32)
    ln2_w = np.asarray(ln2_w, dtype=np.float32)
    Wq = np.asarray(Wq, dtype=np.float32)
    Wk = np.asarray(Wk, dtype=np.float32)
    Wv = np.asarray(Wv, dtype=np.float32)
    Wo = np.asarray(Wo, dtype=np.float32)
    Wg = np.asarray(Wg, dtype=np.float32)
    W1 = np.asarray(W1, dtype=np.float32)
    W2 = np.asarray(W2, dtype=np.float32)
    W3 = np.asarray(W3, dtype=np.float32)
    LAST_EXEC_NS.clear()

    # ---- Phase A ----
    h1 = _rmsnorm_np(x, ln1_w)                       # (B,T,C)
    h1T = np.ascontiguousarray(h1.transpose(0, 2, 1))  # (B,C,T)
    cosT = np.ascontiguousarray(cos.T)
    sinT = np.ascontiguousarray(sin.T)
    Wq_rot = _rot_weights(Wq, H)
    Wk_rot = _rot_weights(Wk, HK)

    nc_a = _build_attn()
    in_maps = []
    for i in range(N_CORES):
        b, g = divmod(i, HK)
        qsl = slice(g * REP * HD, (g + 1) * REP * HD)
        ksl = slice(g * HD, (g + 1) * HD)
        in_maps.append({
            "ht": h1T[b],
            "cosT": cosT,
            "sinT": sinT,
            "wq": np.ascontiguousarray(Wq[:, qsl]),
            "wqr": np.ascontiguousarray(Wq_rot[:, qsl]),
            "wk": np.ascontiguousarray(Wk[:, ksl]),
            "wkr": np.ascontiguousarray(Wk_rot[:, ksl]),
            "wv": np.ascontiguousarray(Wv[:, ksl]),
            "wo3": np.ascontiguousarray(
                Wo[qsl].reshape(REP, HD, C)),
        })
    res_a = _run(nc_a, in_maps, "attn")
    x_attn = x.copy()
    for i in range(N_CORES):
        b = i // HK
        x_attn[b] += res_a[i]["attn_out"]

    # ---- Routing on host ----
    h2 = _rmsnorm_np(x_attn, ln2_w).reshape(-1, C)   # (N,C)
    logits = h2 @ Wg                                  # (N,E)
    part = np.argpartition(-logits, 1, axis=1)[:, :2]
    vals = np.take_along_axis(logits, part, axis=1)
    order = np.argsort(-vals, axis=1, kind="stable")
    top_idx = np.take_along_axis(part, order, axis=1)
    top_vals = np.take_along_axis(vals, order, axis=1)
    ex = np.exp(top_vals - top_vals.max(axis=1, keepdims=True))
    top_w = (ex / ex.sum(axis=1, keepdims=True)).astype(np.float32)

    N = h2.shape[0]
    cw = np.zeros((N, E), np.float32)
    cw[np.arange(N), top_idx[:, 0]] += top_w[:, 0]
    cw[np.arange(N), top_idx[:, 1]] += top_w[:, 1]

    lm = logits - logits.max(axis=1, keepdims=True)
    probs = np.exp(lm)
    probs /= probs.sum(axis=1, keepdims=True)
    usage = probs.mean(axis=0)
    imp = probs.sum(axis=0)
    imp = imp / imp.sum()
    aux = np.float32(E * np.sum(usage * imp))

    # ---- Phase B ----
    idx_e = [np.nonzero(cw[:, e] > 0)[0] for e in range(E)]
    counts = [len(ix) for ix in idx_e]
    cap = max(256, -(-max(counts) // 128) * 128)

    nc_b = _build_moe(cap)
    in_maps = []
    for e in range(E):
        ix = idx_e[e]
        xg = np.zeros((cap, C), np.float32)
        xg[:len(ix)] = h2[ix]
        cwv = np.zeros((cap,), np.float32)
        cwv[:len(ix)] = cw[ix, e]
        in_maps.append({
            "xgT": np.ascontiguousarray(xg.T),
            "cw2": np.ascontiguousarray(cwv.reshape(cap // 128, 128).T),
            "w1r": np.ascontiguousarray(
                W1[e].reshape(C, FF // 128, 128).transpose(1, 0, 2)),
            "w3r": np.ascontiguousarray(
                W3[e].reshape(C, FF // 128, 128).transpose(1, 0, 2)),
            "w2": np.ascontiguousarray(W2[e]),
        })
    res_b = _run(nc_b, in_maps, "moe")

    moe = np.zeros((N, C), np.float32)
    for e in range(E):
        ix = idx_e[e]
        moe[ix] += res_b[e]["moe_out"][:len(ix)]

    out = x_attn + moe.reshape(B, T, C)
    return out, aux


# revision 9
# speedup vs baseline: 1.2216x; 1.2216x over previous
"""Trainium2 Bass kernel for nn_MoETransformerBlock_68728066670794.

Strategy (8 NeuronCores):
  Phase A - attention, sharded by (batch, kv-group): core i -> b=i//4, g=i%4.
    Each core computes its 3 query heads (GQA rep=3) of one batch and the
    partial output projection over its 192 Wo rows. Host sums partials.
    Scores are computed transposed (S^T[k,q]) so no on-device transposes are
    needed; softmax uses exp without max-subtraction (scores are O(1) here)
    and the denominator comes from a ones-column appended to V.
    RoPE: rot(q) = h @ Wq_rot (permuted/negated weights); q and rot(q) are
    produced by ONE matmul with the concatenated stationary [Wq|Wq_rot],
    then split via a ScalarE copy (ACT may shift partition base; DVE not).
  Host - routing (rmsnorm, gate logits, top-2, aux loss) in numpy.
  Phase B - MoE, expert-parallel: core e gets expert e's weights and the
    tokens routed to it (capacity-padded). Computes the gated FFN
    (silu(x@W1) * (x@W3)) @ W2, scaled by combine weights. Host scatter-adds.

All matmuls run as float32r (TF32-like, full PE rate; ~1e-4 rel err).
"""

import os
import sys

sys.path.insert(0, "/opt/trn_rl_repo")

from contextlib import ExitStack

import numpy as np

import concourse.bass as bass
import concourse.tile as tile
from concourse import bacc, mybir
from concourse.bass_utils import run_bass_kernel_spmd
from concourse.masks import make_identity

F32 = mybir.dt.float32
F32R = mybir.dt.float32r
AF = mybir.ActivationFunctionType

B, T, C = 2, 1024, 768
H, HK, HD = 12, 4, 64
REP = H // HK  # 3 query heads per kv head
E, FF = 8, 3072
N_CORES = 8
EPS = 1e-6

_cache = {}

# Collected HW exec times (ns) when KERNEL_TRACE=1 (one entry per launch).
LAST_EXEC_NS = []


def _trace_on():
    return os.environ.get("KERNEL_TRACE", "0") == "1"


def _run(nc, in_maps, label):
    kw = {}
    if _trace_on():
        kw["trace"] = True
    res = run_bass_kernel_spmd(nc, in_maps, list(range(N_CORES)), **kw)
    if _trace_on():
        LAST_EXEC_NS.append((label, res.exec_time_ns))
    return res.results


def _pe_warmup(nc, pool, ps_pool, n=16, tag="proj"):
    """Dense dummy matmuls at program start so the PE HAM un-throttles
    while the real input DMAs are still in flight."""
    zro = pool.tile([128, 512], F32, tag="warmz", name="warmz")
    nc.gpsimd.memset(zro[:], 0.0)
    wt = pool.tile([128, 512], F32R, tag="warm", name="warm")
    nc.scalar.copy(wt[:], zro[:])
    for i in range(n):
        pw = ps_pool.tile([128, 512], F32, tag=tag, name="warmps")
        nc.tensor.matmul(pw[:], wt[:, 0:128], wt[:], start=True, stop=True)


# ---------------------------------------------------------------------------
# Phase A: attention
# ---------------------------------------------------------------------------

def _build_attn():
    if "attn" in _cache:
        return _cache["attn"]
    nc = bacc.Bacc("TRN2", target_bir_lowering=False, debug=False,
                   num_devices=N_CORES)
    ht = nc.dram_tensor("ht", [C, T], F32, kind="ExternalInput").ap()
    cosT = nc.dram_tensor("cosT", [HD, T], F32, kind="ExternalInput").ap()
    sinT = nc.dram_tensor("sinT", [HD, T], F32, kind="ExternalInput").ap()
    # wq2: [128, 6, REP, 128] = per head [Wq | Wq_rot] columns, p-major.
    wq2 = nc.dram_tensor("wq2", [128, C // 128, REP, 2 * HD], F32,
                         kind="ExternalInput").ap()
    # wk2: [128, 6, 128] = [Wk | Wk_rot]
    wk2 = nc.dram_tensor("wk2", [128, C // 128, 2 * HD], F32,
                         kind="ExternalInput").ap()
    wv2 = nc.dram_tensor("wv2", [128, C // 128, HD], F32,
                         kind="ExternalInput").ap()
    wo3 = nc.dram_tensor("wo3", [REP, HD, C], F32, kind="ExternalInput").ap()
    out = nc.dram_tensor("attn_out", [T, C], F32, kind="ExternalOutput").ap()

    NCT = C // 128  # 6 contraction tiles
    NTT = T // 128  # 8 token tiles

    with tile.TileContext(nc, trace_sim=False) as tc, ExitStack() as ctx:
        cst = ctx.enter_context(tc.tile_pool(name="cst", bufs=1))
        sbw = ctx.enter_context(tc.tile_pool(name="sbw", bufs=4))
        sbe = ctx.enter_context(tc.tile_pool(name="sbe", bufs=2))
        ps_proj = ctx.enter_context(
            tc.tile_pool(name="ps_proj", bufs=2, space="PSUM"))
        ps_s = ctx.enter_context(
            tc.tile_pool(name="ps_s", bufs=2, space="PSUM"))
        ps_o = ctx.enter_context(
            tc.tile_pool(name="ps_o", bufs=1, space="PSUM"))

        _pe_warmup(nc, cst, ps_proj)

        hsb = cst.tile([128, NCT, T], F32R, tag="hsb")
        nc.sync.dma_start(
            hsb[:], ht.bitcast(F32R).rearrange("(k p) t -> p k t", p=128))
        cos_sb = cst.tile([HD, T], F32, tag="cos")
        nc.sync.dma_start(cos_sb[:], cosT)
        sin_sb = cst.tile([HD, T], F32, tag="sin")
        nc.sync.dma_start(sin_sb[:], sinT)
        wq_sb = cst.tile([128, NCT, REP, 2 * HD], F32R, tag="wq")
        nc.sync.dma_start(wq_sb[:], wq2.bitcast(F32R))
        wk_sb = cst.tile([128, NCT, 2 * HD], F32R, tag="wk")
        nc.sync.dma_start(wk_sb[:], wk2.bitcast(F32R))
        wv_sb = cst.tile([128, NCT, HD], F32R, tag="wv")
        nc.sync.dma_start(wv_sb[:], wv2.bitcast(F32R))
        wo_sb = []
        for h in range(REP):
            w = cst.tile([HD, C], F32R, tag=f"wo{h}")
            nc.sync.dma_start(w[:], wo3[h].bitcast(F32R))
            wo_sb.append(w)

        ident = cst.tile([128, 128], F32, tag="ident")
        make_identity(nc, ident[:])
        ones_t = cst.tile([128, 1], F32, tag="ones")
        nc.gpsimd.memset(ones_t[:], 1.0)

        # ======== stage 1: projections ========
        # V (natural layout, with ones column)
        vT_sb = cst.tile([HD, T], F32, tag="vT")
        for ch in range(2):
            sl = slice(ch * 512, (ch + 1) * 512)
            psv = ps_proj.tile([HD, 512], F32, tag="proj")
            for c in range(NCT):
                nc.tensor.matmul(psv[:], wv_sb[:, c, :], hsb[:, c, sl],
                                 start=(c == 0), stop=(c == NCT - 1))
            nc.scalar.copy(vT_sb[:, sl], psv[:])
        v1 = []
        for tt in range(NTT):
            pst = ps_proj.tile([128, HD], F32, tag="proj")
            nc.tensor.transpose(
                pst[:], vT_sb[:, tt * 128:(tt + 1) * 128], ident[0:HD, 0:HD])
            vt = cst.tile([128, HD + 1], F32R, tag=f"v1_{tt}")
            nc.scalar.copy(vt[:, 0:HD], pst[:])
            nc.scalar.copy(vt[:, HD:HD + 1], ones_t[:])
            v1.append(vt)

        def rope_pair(ps, dst, sl):
            """ps [128, 512]: rows 0:64 = plain proj, 64:128 = rot proj.
            dst[:, sl] = plain*cos + rot*sin."""
            rot = sbw.tile([HD, 512], F32, tag="rot", bufs=3)
            nc.scalar.copy(rot[:], ps[HD:2 * HD, :])
            t1 = sbw.tile([HD, 512], F32, tag="ropetmp")
            t2 = sbw.tile([HD, 512], F32, tag="ropetmp")
            nc.vector.tensor_mul(t1[:], ps[0:HD, :], cos_sb[:, sl])
            nc.vector.tensor_mul(t2[:], rot[:], sin_sb[:, sl])
            nc.vector.tensor_add(dst[:, sl], t1[:], t2[:])

        # K with rope
        krope = cst.tile([HD, T], F32R, tag="krope")
        for ch in range(2):
            sl = slice(ch * 512, (ch + 1) * 512)
            psk = ps_proj.tile([128, 512], F32, tag="proj")
            for c in range(NCT):
                nc.tensor.matmul(psk[:], wk_sb[:, c, :], hsb[:, c, sl],
                                 start=(c == 0), stop=(c == NCT - 1))
            rope_pair(psk, krope, sl)

        # Q with rope (per head)
        qrope = []
        for h in range(REP):
            qr = cst.tile([HD, T], F32R, tag=f"qrope{h}")
            for ch in range(2):
                sl = slice(ch * 512, (ch + 1) * 512)
                psq = ps_proj.tile([128, 512], F32, tag="proj")
                for c in range(NCT):
                    nc.tensor.matmul(psq[:], wq_sb[:, c, h, :],
                                     hsb[:, c, sl],
                                     start=(c == 0), stop=(c == NCT - 1))
                rope_pair(psq, qr, sl)
            qrope.append(qr)

        # ======== stage 2+3 per head: scores/exp/mask -> O -> normalize ====
        oT = []
        for h in range(REP):
            e_tiles = []
            for i in range(NTT):
                et = sbe.tile([128, T - i * 128], F32R, tag=f"e{i}",
                              name=f"e_{h}_{i}")
                ksl = slice(i * 128, (i + 1) * 128)
                if i < 4:
                    w_lo = 512 - i * 128
                    ps_lo = ps_s.tile([128, w_lo], F32, tag="slo")
                    nc.tensor.matmul(ps_lo[:], krope[:, ksl],
                                     qrope[h][:, i * 128:512],
                                     start=True, stop=True)
                    nc.scalar.activation(et[:, 0:w_lo], ps_lo[:],
                                         AF.Exp, scale=0.125)
                ps_hi = ps_s.tile([128, 512], F32, tag="shi")
                qs = max(512, i * 128)
                nc.tensor.matmul(ps_hi[:, qs - 512:512], krope[:, ksl],
                                 qrope[h][:, qs:1024], start=True, stop=True)
                nc.scalar.activation(et[:, qs - i * 128:T - i * 128],
                                     ps_hi[:, qs - 512:512],
                                     AF.Exp, scale=0.125)
                # causal: keep q >= k within the diagonal block
                nc.gpsimd.affine_select(
                    out=et[:, 0:128], in_=et[:, 0:128],
                    compare_op=mybir.AluOpType.is_ge, fill=0.0,
                    base=0, pattern=[[1, 128]], channel_multiplier=-1)
                e_tiles.append(et)

            pso = ps_o.tile([HD + 1, T], F32, tag="o")
            for i in range(4):
                nc.tensor.matmul(pso[:, i * 128:512], v1[i][:],
                                 e_tiles[i][:, 0:512 - i * 128],
                                 start=(i == 0), stop=(i == 3))
            for i in range(NTT):
                qs = max(512, i * 128)
                nc.tensor.matmul(pso[:, qs:1024], v1[i][:],
                                 e_tiles[i][:, qs - i * 128:T - i * 128],
                                 start=(i == 0), stop=(i == NTT - 1))
            lrow = sbw.tile([1, T], F32, tag="lrow", bufs=2)
            nc.scalar.copy(lrow[:], pso[HD:HD + 1, :])
            rinv = sbw.tile([1, T], F32, tag="rinv", bufs=2)
            nc.vector.reciprocal(rinv[:], lrow[:])
            rlb = sbw.tile([HD, T], F32, tag="rlb", bufs=2)
            nc.gpsimd.partition_broadcast(rlb[:], rinv[:], channels=HD)
            ot = cst.tile([HD, T], F32R, tag=f"oT{h}", name=f"oT{h}")
            for ch in range(2):
                sl = slice(ch * 512, (ch + 1) * 512)
                nc.vector.tensor_mul(ot[:, sl], pso[0:HD, sl], rlb[:, sl])
            oT.append(ot)

        # ======== stage 4: output projection (partial Wo rows) ========
        for tt in range(NTT):
            tsl = slice(tt * 128, (tt + 1) * 128)
            pwa = ps_s.tile([128, 512], F32, tag="slo")
            pwb = ps_s.tile([128, 256], F32, tag="shi")
            for h in range(REP):
                nc.tensor.matmul(pwa[:], oT[h][:, tsl], wo_sb[h][:, 0:512],
                                 start=(h == 0), stop=(h == REP - 1))
            for h in range(REP):
                nc.tensor.matmul(pwb[:], oT[h][:, tsl], wo_sb[h][:, 512:768],
                                 start=(h == 0), stop=(h == REP - 1))
            osa = sbw.tile([128, 512], F32, tag="osa", bufs=3)
            osb = sbw.tile([128, 256], F32, tag="osb", bufs=3)
            nc.scalar.copy(osa[:], pwa[:])
            nc.vector.tensor_copy(osb[:], pwb[:])
            nc.sync.dma_start(out[tsl, 0:512], osa[:])
            nc.sync.dma_start(out[tsl, 512:768], osb[:])

    nc.compile()
    _cache["attn"] = nc
    return nc


# ---------------------------------------------------------------------------
# Phase B: MoE expert FFN
# ---------------------------------------------------------------------------

def _pieces(cap):
    """Decompose cap into free-dim pieces (prefer >=256 for full-rate f32r)."""
    out, r, off = [], cap, 0
    while r > 0:
        if r <= 512:
            ln = r
        elif r - 512 >= 256:
            ln = 512
        else:
            ln = 384
        out.append((off, ln))
        off += ln
        r -= ln
    return out


def _build_moe(cap):
    key = ("moe", cap)
    if key in _cache:
        return _cache[key]
    nc = bacc.Bacc("TRN2", target_bir_lowering=False, debug=False,
                   num_devices=N_CORES)
    xgT = nc.dram_tensor("xgT", [C, cap], F32, kind="ExternalInput").ap()
    cw2 = nc.dram_tensor("cw2", [128, cap // 128], F32,
                         kind="ExternalInput").ap()
    # w1r/w3r: [FF/128, 128, 6, 128] p-major per f-tile (fat DMA lines)
    w1r = nc.dram_tensor("w1r", [FF // 128, 128, C // 128, 128], F32,
                         kind="ExternalInput").ap()
    w3r = nc.dram_tensor("w3r", [FF // 128, 128, C // 128, 128], F32,
                         kind="ExternalInput").ap()
    # w2r: [128, FF/128, 768] p-major
    w2r = nc.dram_tensor("w2r", [128, FF // 128, C], F32,
                         kind="ExternalInput").ap()
    out = nc.dram_tensor("moe_out", [cap, C], F32, kind="ExternalOutput").ap()

    NCT = C // 128   # 6
    NFT = FF // 128  # 24
    NTT = cap // 128
    pieces = _pieces(cap)

    with tile.TileContext(nc, trace_sim=False) as tc, ExitStack() as ctx:
        cst = ctx.enter_context(tc.tile_pool(name="cst", bufs=1))
        sbw = ctx.enter_context(tc.tile_pool(name="sbw", bufs=3))
        psp = ctx.enter_context(tc.tile_pool(name="psp", bufs=2, space="PSUM"))

        _pe_warmup(nc, cst, psp, n=24, tag="p1_0")

        xg_sb = cst.tile([128, NCT, cap], F32R, tag="xg")
        nc.sync.dma_start(
            xg_sb[:], xgT.bitcast(F32R).rearrange("(k p) t -> p k t", p=128))
        cw_sb = cst.tile([128, cap // 128], F32, tag="cw")
        nc.sync.dma_start(cw_sb[:], cw2)
        w2_sb = cst.tile([128, NFT, C], F32R, tag="w2")
        nc.sync.dma_start(w2_sb[:], w2r.bitcast(F32R))

        aT = [cst.tile([128, cap], F32R, tag=f"aT{ft}", name=f"aT{ft}")
              for ft in range(NFT)]

        for ft in range(NFT):
            w1t = sbw.tile([128, NCT, 128], F32R, tag="w1t", bufs=4)
            nc.sync.dma_start(w1t[:], w1r[ft].bitcast(F32R))
            w3t = sbw.tile([128, NCT, 128], F32R, tag="w3t", bufs=4)
            nc.sync.dma_start(w3t[:], w3r[ft].bitcast(F32R))
            for pi, (poff, plen) in enumerate(pieces):
                psl = slice(poff, poff + plen)
                ps1 = psp.tile([128, plen], F32, tag=f"p1_{pi}")
                ps3 = psp.tile([128, plen], F32, tag=f"p3_{pi}")
                for c in range(NCT):
                    nc.tensor.matmul(ps1[:], w1t[:, c, :], xg_sb[:, c, psl],
                                     start=(c == 0), stop=(c == NCT - 1))
                for c in range(NCT):
                    nc.tensor.matmul(ps3[:], w3t[:, c, :], xg_sb[:, c, psl],
                                     start=(c == 0), stop=(c == NCT - 1))
                ss = sbw.tile([128, plen], F32, tag="silu", bufs=4)
                nc.scalar.activation(ss[:], ps1[:], AF.Silu)
                nc.vector.tensor_mul(aT[ft][:, psl], ss[:], ps3[:])

        for ct in range(NTT):
            tsl = slice(ct * 128, (ct + 1) * 128)
            pya = psp.tile([128, 512], F32, tag="p1_0")
            pyb = psp.tile([128, 256], F32, tag="p3_0")
            for ft in range(NFT):
                nc.tensor.matmul(pya[:], aT[ft][:, tsl], w2_sb[:, ft, 0:512],
                                 start=(ft == 0), stop=(ft == NFT - 1))
            for ft in range(NFT):
                nc.tensor.matmul(pyb[:], aT[ft][:, tsl], w2_sb[:, ft, 512:768],
                                 start=(ft == 0), stop=(ft == NFT - 1))
            ya = sbw.tile([128, 512], F32, tag="yout")
            yb = sbw.tile([128, 256], F32, tag="youtb")
            nc.vector.tensor_scalar_mul(ya[:], pya[:], cw_sb[:, ct:ct + 1])
            nc.vector.tensor_scalar_mul(yb[:], pyb[:], cw_sb[:, ct:ct + 1])
            nc.sync.dma_start(out[tsl, 0:512], ya[:])
            nc.sync.dma_start(out[tsl, 512:768], yb[:])

    nc.compile()
    _cache[key] = nc
    return nc


# ---------------------------------------------------------------------------
# Host orchestration
# ---------------------------------------------------------------------------

def _rmsnorm_np(x, w):
    ms = np.mean(x.astype(np.float64) ** 2, axis=-1, keepdims=True)
    return (x * (1.0 / np.sqrt(ms + EPS))).astype(np.float32) * w


def _rot_weights(w, n_heads):
    """wr such that h @ wr == rotate_half(h @ w) per head of width HD."""
    wr = np.empty_like(w)
    for j in range(n_heads):
        c = j * HD
        wr[:, c:c + HD // 2] = -w[:, c + HD // 2:c + HD]
        wr[:, c + HD // 2:c + HD] = w[:, c:c + HD // 2]
    return wr


def _pmajor(w):
    """[C, D] -> [128, C/128, D] (partition-major, fat DMA lines)."""
    ck, d = w.shape
    return np.ascontiguousarray(
        w.reshape(ck // 128, 128, d).transpose(1, 0, 2))


def kernel(x, cos, sin, ln1_w, ln2_w, Wq, Wk, Wv, Wo, Wg, W1, W2, W3):
    x = np.asarray(x, dtype=np.float32)
    cos = np.asarray(cos, dtype=np.float32)
    sin = np.asarray(sin, dtype=np.float32)
    ln1_w = np.asarray(ln1_w, dtype=np.float32)
    ln2_w = np.asarray(ln2_w, dtype=np.float32)
    Wq = np.asarray(Wq, dtype=np.float32)
    Wk = np.asarray(Wk, dtype=np.float32)
    Wv = np.asarray(Wv, dtype=np.float32)
    Wo = np.asarray(Wo, dtype=np.float32)
    Wg = np.asarray(Wg, dtype=np.float32)
    W1 = np.asarray(W1, dtype=np.float32)
    W2 = np.asarray(W2, dtype=np.float32)
    W3 = np.asarray(W3, dtype=np.float32)
    LAST_EXEC_NS.clear()

    # ---- Phase A ----
    h1 = _rmsnorm_np(x, ln1_w)                         # (B,T,C)
    h1T = np.ascontiguousarray(h1.transpose(0, 2, 1))  # (B,C,T)
    cosT = np.ascontiguousarray(cos.T)
    sinT = np.ascontiguousarray(sin.T)
    Wq_rot = _rot_weights(Wq, H)
    Wk_rot = _rot_weights(Wk, HK)

    nc_a = _build_attn()
    in_maps = []
    for i in range(N_CORES):
        b, g = divmod(i, HK)
        qsl = slice(g * REP * HD, (g + 1) * REP * HD)
        ksl = slice(g * HD, (g + 1) * HD)
        # per head [Wq | Wq_rot]: [C, REP, 128]
        wq_pair = np.stack([
            np.concatenate(
                [Wq[:, qsl][:, h * HD:(h + 1) * HD],
                 Wq_rot[:, qsl][:, h * HD:(h + 1) * HD]], axis=1)
            for h in range(REP)], axis=1)          # [C, REP, 128]
        wq2 = _pmajor(wq_pair.reshape(C, REP * 2 * HD)).reshape(
            128, C // 128, REP, 2 * HD)
        wk_pair = np.concatenate([Wk[:, ksl], Wk_rot[:, ksl]], axis=1)
        in_maps.append({
            "ht": h1T[b],
            "cosT": cosT,
            "sinT": sinT,
            "wq2": np.ascontiguousarray(wq2),
            "wk2": _pmajor(wk_pair),
            "wv2": _pmajor(np.ascontiguousarray(Wv[:, ksl])),
            "wo3": np.ascontiguousarray(Wo[qsl].reshape(REP, HD, C)),
        })
    res_a = _run(nc_a, in_maps, "attn")
    x_attn = x.copy()
    for i in range(N_CORES):
        b = i // HK
        x_attn[b] += res_a[i]["attn_out"]

    # ---- Routing on host ----
    h2 = _rmsnorm_np(x_attn, ln2_w).reshape(-1, C)   # (N,C)
    logits = h2 @ Wg                                  # (N,E)
    part = np.argpartition(-logits, 1, axis=1)[:, :2]
    vals = np.take_along_axis(logits, part, axis=1)
    order = np.argsort(-vals, axis=1, kind="stable")
    top_idx = np.take_along_axis(part, order, axis=1)
    top_vals = np.take_along_axis(vals, order, axis=1)
    ex = np.exp(top_vals - top_vals.max(axis=1, keepdims=True))
    top_w = (ex / ex.sum(axis=1, keepdims=True)).astype(np.float32)

    N = h2.shape[0]
    cw = np.zeros((N, E), np.float32)
    cw[np.arange(N), top_idx[:, 0]] += top_w[:, 0]
    cw[np.arange(N), top_idx[:, 1]] += top_w[:, 1]

    lm = logits - logits.max(axis=1, keepdims=True)
    probs = np.exp(lm)
    probs /= probs.sum(axis=1, keepdims=True)
    usage = probs.mean(axis=0)
    imp = probs.sum(axis=0)
    imp = imp / imp.sum()
    aux = np.float32(E * np.sum(usage * imp))

    # ---- Phase B ----
    idx_e = [np.nonzero(cw[:, e] > 0)[0] for e in range(E)]
    counts = [len(ix) for ix in idx_e]
    cap = max(256, -(-max(counts) // 128) * 128)

    nc_b = _build_moe(cap)
    in_maps = []
    for e in range(E):
        ix = idx_e[e]
        xg = np.zeros((cap, C), np.float32)
        xg[:len(ix)] = h2[ix]
        cwv = np.zeros((cap,), np.float32)
        cwv[:len(ix)] = cw[ix, e]
        w1p = _pmajor(W1[e]).reshape(128, C // 128, FF // 128, 128)
        w3p = _pmajor(W3[e]).reshape(128, C // 128, FF // 128, 128)
        in_maps.append({
            "xgT": np.ascontiguousarray(xg.T),
            "cw2": np.ascontiguousarray(cwv.reshape(cap // 128, 128).T),
            "w1r": np.ascontiguousarray(w1p.transpose(2, 0, 1, 3)),
            "w3r": np.ascontiguousarray(w3p.transpose(2, 0, 1, 3)),
            "w2r": _pmajor(W2[e]),
        })
    res_b = _run(nc_b, in_maps, "moe")

    moe = np.zeros((N, C), np.float32)
    for e in range(E):
        ix = idx_e[e]
        moe[ix] += res_b[e]["moe_out"][:len(ix)]

    out = x_attn + moe.reshape(B, T, C)
    return out, aux
